# revision 1
# baseline (speedup 1.0000x reference)
"""Compound loss (dice + focal + edge) kernel for Trainium2, 8-core data-parallel.

Shapes hardcoded: inputs [8, 11, 512, 512] f32, targets [8, 512, 512] int.
Each NeuronCore processes one batch sample and emits per-class partial sums
(stats [128, 268] f32); the tiny cross-batch combination happens on host.

Design notes:
- row-tiles of 128 rows; layout [h=partition, (c,w)=free].
- softmax probs P = exp(X) * (1/sum_c exp(X)) without max-subtraction (randn).
- onehot/pred one-hot handled as dense [128,11,512] f32 tiles.
- 3x3 convs on the class-mask bit-words (m = 1<<t): separable OR/AND built on
  GPSIMD, vertical shifts via a DRAM round-trip of the word planes.
- per-class sums via ScalarE (ACT) Copy+accum_out; focal via plane reduce.
- tensor_tensor_reduce is avoided entirely (crashes HW via this runtime).
"""

import sys

sys.path.insert(0, "/opt/trn_rl_repo")

import functools
import numpy as np

B, C, H, W = 8, 11, 512, 512
P = 128
NT = H // P
EPS = 1e-6
FOCAL_ALPHA = 0.25
E1 = float(np.exp(-1.0))
ES = float(np.exp(-np.sqrt(2.0)))

PER_TILE = 6 * C + 1
NCOL = NT * PER_TILE  # 268


def _col(k, q, c=0):
    base = k * PER_TILE
    if q == 6:
        return base + 6 * C
    return base + q * C + c


@functools.cache
def _build():
    import concourse.bacc as bacc
    from concourse import mybir, tile

    f32 = mybir.dt.float32
    i32 = mybir.dt.int32
    A = mybir.AluOpType
    AF = mybir.ActivationFunctionType
    AX = mybir.AxisListType.X

    nc = bacc.Bacc(None, target_bir_lowering=False)
    xin = nc.dram_tensor("inputs", [C, H, W], f32, kind="ExternalInput")
    tin = nc.dram_tensor("targets", [H, W], i32, kind="ExternalInput")
    out = nc.dram_tensor("stats", [P, NCOL], f32, kind="ExternalOutput")

    with tile.TileContext(nc) as tc:
        with (
            tc.tile_pool(name="const", bufs=1) as cpool,
            tc.tile_pool(name="dram", bufs=1, space="DRAM") as dpool,
            tc.tile_pool(name="big", bufs=1) as bpool,
            tc.tile_pool(name="xbuf", bufs=2) as xpool,
            tc.tile_pool(name="pl", bufs=1) as pp,
            tc.tile_pool(name="w2", bufs=2) as pp2,
        ):
            ones_i = cpool.tile([P, W], i32)
            nc.vector.memset(ones_i[:], 1)
            stats = cpool.tile([P, NCOL], f32)

            d_mw = dpool.tile([H, W], i32)
            d_rw = dpool.tile([H, W], i32)
            d_ro = dpool.tile([H, W], i32)
            d_ar = dpool.tile([H, W], i32)

            # ---------- phase 1: word planes -> DRAM
            Ts = []
            for k in range(NT):
                h0 = k * P
                T = cpool.tile([P, W], i32, tag=f"T{k}")
                nc.sync.dma_start(T[:], tin[h0 : h0 + P, :])
                Ts.append(T)

                mw = pp.tile([P, W], i32, tag="p1mw")
                nc.vector.tensor_tensor(mw[:], ones_i[:], T[:], A.logical_shift_left)
                rw = pp.tile([P, W], i32, tag="p1rw")
                nc.vector.memset(rw[:, 0:1], 0)
                nc.vector.tensor_copy(rw[:, 1:W], mw[:, 0 : W - 1])
                nc.vector.tensor_tensor(
                    rw[:, 0 : W - 1], rw[:, 0 : W - 1], mw[:, 1:W], A.bitwise_or
                )
                ro = pp.tile([P, W], i32, tag="p1ro")
                nc.vector.tensor_tensor(ro[:], rw[:], mw[:], A.bitwise_or)
                ar = pp.tile([P, W], i32, tag="p1ar")
                nc.vector.memset(ar[:, 0:1], 0)
                nc.vector.memset(ar[:, W - 1 : W], 0)
                nc.vector.tensor_tensor(
                    ar[:, 1 : W - 1], mw[:, 1 : W - 1], mw[:, 0 : W - 2], A.bitwise_and
                )
                nc.vector.tensor_tensor(
                    ar[:, 1 : W - 1], ar[:, 1 : W - 1], mw[:, 2:W], A.bitwise_and
                )
                nc.sync.dma_start(d_mw[h0 : h0 + P, :], mw[:])
                nc.sync.dma_start(d_rw[h0 : h0 + P, :], rw[:])
                nc.sync.dma_start(d_ro[h0 : h0 + P, :], ro[:])
                nc.sync.dma_start(d_ar[h0 : h0 + P, :], ar[:])

            # helper: load rows [h0+off .. h0+off+127] of a DRAM plane, zero-pad OOB
            def vload(dst, dplane, h0, off):
                lo = h0 + off
                if lo < 0:
                    nc.vector.memset(dst[:], 0)
                    nc.sync.dma_start(dst[1:P, :], dplane[0 : P - 1, :])
                elif lo + P > H:
                    nc.vector.memset(dst[:], 0)
                    nc.sync.dma_start(dst[0 : P - 1, :], dplane[lo : H, :])
                else:
                    nc.sync.dma_start(dst[:], dplane[lo : lo + P, :])

            # ---------- phase 2
            for k in range(NT):
                h0 = k * P
                T = Ts[k]

                Xt = xpool.tile([P, C, W], f32, tag="X")
                nc.sync.dma_start(
                    Xt[:], xin[:, h0 : h0 + P, :].rearrange("c h w -> h c w")
                )

                E = bpool.tile([P, C, W], f32, tag="E")
                nc.scalar.activation(E[:], Xt[:], AF.Exp)
                Dn = pp.tile([P, W], f32, tag="Dn")
                nc.vector.reduce_sum(Dn[:], E[:].transpose([0, 2, 1]), axis=AX)
                r = pp.tile([P, W], f32, tag="r")
                nc.vector.reciprocal(r[:], Dn[:])
                # P = E * r  (in place: E now holds probs)
                nc.vector.tensor_tensor(
                    E[:], E[:], r[:].unsqueeze(1).broadcast_to([P, C, W]), A.mult
                )
                Pr = E

                OH = bpool.tile([P, C, W], f32, tag="OH")
                for c in range(C):
                    nc.vector.tensor_scalar(OH[:, c, :], T[:], c, None, A.is_equal)

                # Q = OH * P (into X slot; X dead after exp)
                Q = Xt
                nc.vector.tensor_tensor(Q[:], OH[:], Pr[:], A.mult)
                pt = pp.tile([P, W], f32, tag="pt")
                nc.vector.reduce_sum(pt[:], Q[:].transpose([0, 2, 1]), axis=AX)

                Pmax = pp.tile([P, W], f32, tag="Pmax")
                nc.vector.reduce_max(Pmax[:], Pr[:].transpose([0, 2, 1]), axis=AX)
                PRED = bpool.tile([P, C, W], f32, tag="PRED")
                nc.vector.tensor_tensor(
                    PRED[:], Pr[:], Pmax[:].unsqueeze(1).broadcast_to([P, C, W]),
                    A.is_equal,
                )
                npe = pp.tile([P, W], f32, tag="npe")
                nc.vector.tensor_tensor(npe[:], pt[:], Pmax[:], A.is_equal)
                nc.vector.tensor_scalar(npe[:], npe[:], -1.0, 1.0, A.mult, A.add)

                # per-class sums: soh, inter, sumP (ScalarE accum)
                scrA = pp.tile([P, W], f32, tag="scrA")
                scrB = pp.tile([P, W], f32, tag="scrB")
                scrC = pp.tile([P, W], f32, tag="scrC")
                for c in range(C):
                    nc.scalar.activation(
                        scrA[:], OH[:, c, :], AF.Copy,
                        accum_out=stats[:, _col(k, 0, c) : _col(k, 0, c) + 1],
                    )
                    nc.scalar.activation(
                        scrB[:], Q[:, c, :], AF.Copy,
                        accum_out=stats[:, _col(k, 1, c) : _col(k, 1, c) + 1],
                    )
                    nc.scalar.activation(
                        scrC[:], Pr[:, c, :], AF.Copy,
                        accum_out=stats[:, _col(k, 2, c) : _col(k, 2, c) + 1],
                    )

                # PWQ = PRED * 2^c (in place, ScalarE)
                for c in range(C):
                    nc.scalar.activation(
                        PRED[:, c, :], PRED[:, c, :], AF.Copy, scale=float(1 << c)
                    )
                PWQ = PRED
                pmf = pp.tile([P, W], f32, tag="pmf")
                nc.vector.reduce_sum(pmf[:], PWQ[:].transpose([0, 2, 1]), axis=AX)
                pmi = pp.tile([P, W], i32, tag="pmi")
                nc.vector.tensor_copy(pmi[:], pmf[:])

                # focal
                nc.vector.tensor_scalar_max(pt[:], pt[:], 1e-7)
                Lp = pp.tile([P, W], f32, tag="Lp")
                nc.scalar.activation(Lp[:], pt[:], AF.Ln)
                u2 = pp.tile([P, W], f32, tag="u2")
                nc.scalar.activation(u2[:], pt[:], AF.Square, bias=1.0, scale=-1.0)
                nc.vector.tensor_tensor(u2[:], u2[:], Lp[:], A.mult)
                nc.vector.reduce_sum(
                    stats[:, _col(k, 6) : _col(k, 6) + 1], u2[:], axis=AX
                )

                # words: or8 / an9 / or4 from DRAM planes
                vu = pp2.tile([P, W], i32, tag="vu")
                vd = pp2.tile([P, W], i32, tag="vd")
                vc = pp2.tile([P, W], i32, tag="vc")
                or8 = pp.tile([P, W], i32, tag="or8")
                vload(vc, d_ro, h0, 0)
                vload(vu, d_ro, h0, -1)
                vload(vd, d_ro, h0, 1)
                nc.vector.tensor_tensor(or8[:], vc[:], vu[:], A.bitwise_or)
                nc.vector.tensor_tensor(or8[:], or8[:], vd[:], A.bitwise_or)

                an9 = pp.tile([P, W], i32, tag="an9")
                vu2 = pp2.tile([P, W], i32, tag="vu")
                vd2 = pp2.tile([P, W], i32, tag="vd")
                vc2 = pp2.tile([P, W], i32, tag="vc")
                vload(vc2, d_ar, h0, 0)
                vload(vu2, d_ar, h0, -1)
                vload(vd2, d_ar, h0, 1)
                nc.vector.tensor_tensor(an9[:], vc2[:], vu2[:], A.bitwise_and)
                nc.vector.tensor_tensor(an9[:], an9[:], vd2[:], A.bitwise_and)

                or4 = pp.tile([P, W], i32, tag="or4")
                mwc = pp.tile([P, W], i32, tag="mwc")
                vu3 = pp2.tile([P, W], i32, tag="vu")
                vd3 = pp2.tile([P, W], i32, tag="vd")
                vload(mwc, d_mw, h0, 0)
                vload(vu3, d_mw, h0, -1)
                vload(vd3, d_mw, h0, 1)
                vc3 = pp2.tile([P, W], i32, tag="vc")
                vload(vc3, d_rw, h0, 0)
                nc.vector.tensor_tensor(or4[:], vc3[:], vu3[:], A.bitwise_or)
                nc.vector.tensor_tensor(or4[:], or4[:], vd3[:], A.bitwise_or)

                # b9t before an9 is overwritten by BW
                b9t = pp.tile([P, W], f32, tag="b9t")
                nc.vector.tensor_tensor(b9t[:], an9[:], mwc[:], A.is_equal)
                # BW = ~an9 & or8  (in place into an9)
                nc.vector.tensor_scalar(an9[:], an9[:], -1, None, A.bitwise_xor)
                nc.vector.tensor_tensor(an9[:], an9[:], or8[:], A.bitwise_and)
                BW = an9

                # gA plane = npe * (1 - b9t)
                nc.vector.tensor_scalar(b9t[:], b9t[:], -1.0, 1.0, A.mult, A.add)
                gAp = pp.tile([P, W], f32, tag="gAp")
                nc.vector.tensor_tensor(gAp[:], npe[:], b9t[:], A.mult)

                # B0p / O4p bit-gathers
                ti = pp.tile([P, W], i32, tag="ti")
                nc.vector.tensor_tensor(ti[:], or8[:], pmi[:], A.bitwise_and)
                B0p = pp.tile([P, W], f32, tag="B0p")
                nc.vector.tensor_scalar(B0p[:], ti[:], 0, None, A.is_gt)
                ti2 = pp.tile([P, W], i32, tag="ti2")
                nc.vector.tensor_tensor(ti2[:], or4[:], pmi[:], A.bitwise_and)
                O4p = pp.tile([P, W], f32, tag="O4p")
                nc.vector.tensor_scalar(O4p[:], ti2[:], 0, None, A.is_gt)

                # g23 = (ES + (E1-ES)*O4p) * B0p * npe
                g23 = pp.tile([P, W], f32, tag="g23")
                nc.vector.tensor_scalar(g23[:], O4p[:], E1 - ES, ES, A.mult, A.add)
                nc.vector.tensor_tensor(g23[:], g23[:], B0p[:], A.mult)
                nc.vector.tensor_tensor(g23[:], g23[:], npe[:], A.mult)

                # GR = OH * gA (in place), NR = PWQ * g23 (in place) on GPSIMD
                nc.gpsimd.tensor_tensor(
                    OH[:], OH[:], gAp[:].unsqueeze(1).broadcast_to([P, C, W]), A.mult
                )
                nc.gpsimd.tensor_tensor(
                    PWQ[:], PWQ[:], g23[:].unsqueeze(1).broadcast_to([P, C, W]), A.mult
                )
                scrD = pp.tile([P, W], f32, tag="scrD")
                scrE = pp.tile([P, W], f32, tag="scrE")
                for c in range(C):
                    nc.scalar.activation(
                        scrD[:], OH[:, c, :], AF.Copy,
                        accum_out=stats[:, _col(k, 3, c) : _col(k, 3, c) + 1],
                    )
                    nc.scalar.activation(
                        scrE[:], PWQ[:, c, :], AF.Copy,
                        accum_out=stats[:, _col(k, 4, c) : _col(k, 4, c) + 1],
                    )

                # ne: (BW & 2^c) summed (ScalarE accum, int->f32)
                for c in range(C):
                    scri = pp2.tile([P, W], i32, tag="scri")
                    scrF = pp2.tile([P, W], f32, tag="scrF")
                    nc.vector.tensor_scalar(
                        scri[:], BW[:], 1 << c, None, A.bitwise_and
                    )
                    nc.scalar.activation(
                        scrF[:], scri[:], AF.Copy,
                        accum_out=stats[:, _col(k, 5, c) : _col(k, 5, c) + 1],
                    )

            nc.sync.dma_start(out[:], stats[:])

    nc.compile()
    return nc


def _host_combine(stats_list):
    soh = np.zeros((B, C)); inter = np.zeros((B, C)); sumP = np.zeros((B, C))
    gA = np.zeros((B, C)); n23 = np.zeros((B, C)); ne = np.zeros((B, C))
    fsum = np.zeros(B)
    pw2 = 2.0 ** np.arange(C)
    for b in range(B):
        st = stats_list[b].astype(np.float64).sum(axis=0)
        for k in range(NT):
            soh[b] += st[_col(k, 0) : _col(k, 0) + C]
            inter[b] += st[_col(k, 1) : _col(k, 1) + C]
            sumP[b] += st[_col(k, 2) : _col(k, 2) + C]
            gA[b] += st[_col(k, 3) : _col(k, 3) + C]
            n23[b] += st[_col(k, 4) : _col(k, 4) + C] / pw2
            ne[b] += st[_col(k, 5) : _col(k, 5) + C] / pw2
            fsum[b] += st[_col(k, 6)]

    dice = (2.0 * inter + EPS) / (sumP + soh + EPS)
    cls = np.arange(C)
    cls_valid = (soh.sum(axis=0) > 0) & (cls != 0)
    nvalid = int(cls_valid.sum())
    dice_score = (dice.mean(axis=0) * cls_valid).sum() / max(nvalid, 1)
    dice_loss = (1.0 - dice_score) if nvalid > 0 else 0.0

    focal_loss = -FOCAL_ALPHA * fsum.sum() / (B * H * W)

    werr = gA + n23
    class_loss = werr / np.maximum(ne, 1.0)
    valid_bc = (soh > 0) & (cls[None, :] != 0)
    nvalid_b = valid_bc.sum(axis=1)
    sample = (class_loss * valid_bc).sum(axis=1) / np.maximum(nvalid_b, 1)
    edge_loss = float(np.where(nvalid_b > 0, sample, 0.0).mean())

    total = dice_loss + focal_loss + edge_loss
    return (
        np.float32(total),
        np.float32(dice_loss),
        np.float32(focal_loss),
        np.float32(edge_loss),
    )


def kernel(inputs, targets):
    from concourse.bass_utils import run_bass_kernel_spmd

    inputs = np.ascontiguousarray(np.asarray(inputs, dtype=np.float32))
    tgt = np.ascontiguousarray(np.asarray(targets).astype(np.int32))

    nc = _build()
    in_maps = [{"inputs": inputs[b], "targets": tgt[b]} for b in range(B)]
    res = run_bass_kernel_spmd(nc, in_maps, core_ids=list(range(B)))
    return _host_combine([res.results[b]["stats"] for b in range(B)])

